# revision 20
# baseline (speedup 1.0000x reference)
"""Depthwise causal Conv1d (k=4) + SiLU on 8 Trainium2 NeuronCores.

Problem: x [4, 4096, 2048] f32, w [2048, 4] f32,
out[b, t, d] = silu(sum_j w[d, j] * x[b, t - 3 + j, d])   (zero-padded left).

Sharding: 8 cores = 4 batches x 2 channel-halves. Depthwise conv is
independent per channel, so channel sharding needs no halo exchange.

Layout: each core receives its shard host-transposed to [channels, time]
(channels on SBUF partitions). The per-channel weight w[d, j] is a
per-partition scalar and the causal time shifts are free-dim AP offsets
into one loaded tile.

The kernel is HBM-bandwidth-bound (~16.8 MB of fp16 I/O per core against
~390 GB/s effective per-NeuronCore HBM), with compute only just fitting
under the DMA window, so the schedule works both angles:
 - All 8 input-block loads are queued back to back at the head of the
   sync HWDGE ring; stores are issued on the SAME ring afterwards. The
   ring drains FIFO, which gives loads strict priority over stores
   (loads finish at ~30 us) and keeps the SDMA engines fed to the end.
 - Compute is spread over FOUR engine lanes so no engine exceeds the
   DMA window: odd blocks run on the TensorEngine as diag(w_j) matmuls
   accumulating the 4 taps in PSUM (diag built on-chip from a 32 KB
   host-sent identity); even blocks run elementwise with the products
   split between DVE and GpSimd (GpSimd is otherwise idle since stores
   left the SWDGE path) and the pair-add tree on DVE. ACT does only
   SiLU.

Measured op rates (fp16): DVE tensor_scalar 0.29 ns/elem, DVE
tensor_tensor 0.54, GpSimd tensor_scalar ~0.9, ACT ~1.0, PE ~9.3 us per
4-tap 4096-col block.

Precision: x and the output are host-cast fp16 (halves HBM traffic both
ways); products and adds stay fp16 (PE accumulates fp32 in PSUM); SiLU
computes fp32-internally on ACT. End-to-end relative error ~5e-4.
"""

import sys
import types

import numpy as np

import concourse.bass as bass
import concourse.bacc as bacc
import concourse.mybir as mybir
from concourse.tile import TileContext
from concourse.bass_utils import run_bass_kernel_spmd


def _ensure_ntff_hook():
    """bass_utils imports antenv.axon_hooks when BASS_TRACE is set; that
    module is absent on this image. Install a shim so tracing works when
    possible and degrades gracefully (instead of crashing) when not."""
    try:
        import antenv.axon_hooks  # noqa: F401

        return
    except ImportError:
        pass
    try:
        import antenv

        hook = None
        try:
            if "/root/.axon_site" not in sys.path:
                sys.path.insert(0, "/root/.axon_site")
            from trn_agent_boot.trn_boot import _ntff_profile_via_ctypes

            hook = _ntff_profile_via_ctypes("/opt/axon/libaxon_pjrt.so")
        except Exception:
            hook = None
        mod = types.ModuleType("antenv.axon_hooks")
        mod._hook = hook
        mod.get_axon_ntff_profile_hook = lambda: mod._hook
        mod.set_axon_ntff_profile_hook = lambda h: setattr(mod, "_hook", h)
        sys.modules["antenv.axon_hooks"] = mod
        antenv.axon_hooks = mod
    except Exception:
        pass


_ensure_ntff_hook()

B, L, D = 4, 4096, 2048
K = 4
PAD = K - 1
N_CORES = 8
DH = D // 2            # channels per core
NBLK = DH // 128       # 128-partition channel blocks per core
ROWW = 4128            # DRAM row stride (fp16 elems): 64B-aligned rows

MID_DT = mybir.dt.float16
PE_BLKS = (1, 3, 5, 7)  # blocks computed fully on the TensorEngine
PE_HALF_BLK = 6         # block 6: first 2048 cols on DVE, rest on the PE
PE_DIAG_BLKS = (1, 3, 5, 6, 7)  # DRAM order of per-block diag(w_j) slabs

_cache = {}


def _build_bass():
    nc = bacc.Bacc()
    xt = nc.dram_tensor("xt", [DH, ROWW], MID_DT, kind="ExternalInput")
    wt = nc.dram_tensor("wt", [128, NBLK * K], mybir.dt.float32, kind="ExternalInput")
    # host-built diag(w_j) blocks for the PE path, one [K*128] slab per
    # PE block in PE_DIAG_BLKS order
    wd = nc.dram_tensor(
        "wd", [128, len(PE_DIAG_BLKS) * K * 128], MID_DT, kind="ExternalInput"
    )
    ot = nc.dram_tensor("ot", [DH, L], MID_DT, kind="ExternalOutput")
    f32 = mybir.dt.float32
    HALF = L // 2

    with TileContext(nc) as tc:
        with tc.tile_pool(name="pool", bufs=2) as pool, \
             tc.tile_pool(name="psum", bufs=2, space="PSUM") as psum_pool:
            # w leads the sync ring so compute can start as soon as the
            # first x block lands.
            w = pool.tile([128, NBLK * K], f32, tag="w", bufs=1)
            nc.sync.dma_start(out=w[:], in_=wt[:, :])
            # Warmup: a tiny Silu forces the silu activation-table set to
            # load during the initial DMA wait; it is the only table load
            # in the whole kernel.
            warm = pool.tile([128, 2], MID_DT, tag="warm", bufs=1)
            nc.vector.memset(warm[:], 0.0)
            nc.scalar.activation(warm[:], warm[:], mybir.ActivationFunctionType.Silu)

            # All loads up front, back to back on the sync ring. The diag
            # slab for each PE block is interleaved right after that
            # block's x so it lands just in time (DMA has slack; DVE/ACT
            # do not, so the diag is host-built rather than built on-chip).
            xts = []
            wdts = {}
            for blk in range(NBLK):
                x = pool.tile([128, L + PAD + 1], MID_DT, tag="x", bufs=NBLK)
                nc.sync.dma_start(
                    out=x[:, 0 : L + PAD],
                    in_=xt[blk * 128 : (blk + 1) * 128, 0 : L + PAD],
                )
                xts.append(x)
                if blk in PE_DIAG_BLKS:
                    wslab = pool.tile(
                        [128, K * 128], MID_DT, tag="wd", bufs=len(PE_DIAG_BLKS)
                    )
                    c = PE_DIAG_BLKS.index(blk) * K * 128
                    nc.sync.dma_start(out=wslab[:], in_=wd[:, c : c + K * 128])
                    wdts[blk] = wslab

            PQ = 1024

            def pe_part(blk, x, o, t0, size):
                # TensorEngine path for [t0, t0+size): accumulate the 4
                # diag(w_j) matmuls per 512-col PSUM chunk (shift =
                # free-dim offset on the moving operand), SiLU from PSUM.
                # 1024-col PSUM quarters, 4 deep, so the PE stays 3 ahead
                # of ACT's PSUM drain.
                for q0 in range(t0, t0 + size, PQ):
                    ps = psum_pool.tile([128, PQ], f32, tag="ps", bufs=4)
                    for j in range(K):
                        lw = wdts[blk][:, j * 128 : (j + 1) * 128]
                        for cc in range(PQ // 512):
                            nc.tensor.matmul(
                                ps[:, cc * 512 : (cc + 1) * 512],
                                lw,
                                x[:, q0 + cc * 512 + j : q0 + cc * 512 + j + 512],
                                start=(j == 0),
                                stop=(j == K - 1),
                            )
                    nc.scalar.activation(
                        o[:, q0 : q0 + PQ], ps[:], mybir.ActivationFunctionType.Silu
                    )

            def dve_part(blk, x, o, t0, size, n_act=0):
                # Elementwise path for [t0, t0+size): qe holds the
                # even-shift products [q0 | q2], qo the odd [q1 | q3]
                # (n_act of the odd ones on ACT to relieve DVE), pair-add
                # + final add on DVE, then SiLU. Shift-rebased:
                # q_j[:, t] = w_j * x[:, t + j].
                wj = lambda j: w[:, blk * K + j : blk * K + j + 1]
                qe = pool.tile([128, 2, size], MID_DT, tag=f"qe{size}", bufs=3)
                qo = pool.tile([128, 2, size], MID_DT, tag=f"qo{size}", bufs=3)
                if n_act >= 1:
                    nc.scalar.mul(qo[:, 0, :], x[:, t0 + 1 : t0 + 1 + size], wj(1))
                else:
                    nc.vector.tensor_scalar_mul(
                        qo[:, 0, :], x[:, t0 + 1 : t0 + 1 + size], wj(1)
                    )
                nc.vector.tensor_scalar_mul(qe[:, 0, :], x[:, t0 : t0 + size], wj(0))
                if n_act >= 2:
                    nc.scalar.mul(qo[:, 1, :], x[:, t0 + 3 : t0 + 3 + size], wj(3))
                else:
                    nc.vector.tensor_scalar_mul(
                        qo[:, 1, :], x[:, t0 + 3 : t0 + 3 + size], wj(3)
                    )
                nc.vector.tensor_scalar_mul(qe[:, 1, :], x[:, t0 + 2 : t0 + 2 + size], wj(2))
                nc.vector.tensor_add(qe[:, :, :], qe[:, :, :], qo[:, :, :])
                nc.vector.tensor_add(qe[:, 0, :], qe[:, 0, :], qe[:, 1, :])
                nc.scalar.activation(
                    o[:, t0 : t0 + size], qe[:, 0, :],
                    mybir.ActivationFunctionType.Silu,
                )

            # odd products on ACT per (blk, half): balances DVE ~40 / ACT ~40
            ACT_ODD = {(0, 0): 2, (0, 1): 1}
            mult = mybir.AluOpType.mult
            add = mybir.AluOpType.add
            outs = []
            for blk in range(NBLK):
                x = xts[blk]
                o = pool.tile([128, L], MID_DT, tag="o", bufs=NBLK)
                for h in range(2):
                    h0 = h * HALF
                    if blk in PE_BLKS or (blk == PE_HALF_BLK and h == 1):
                        pe_part(blk, x, o, h0, HALF)
                    else:
                        dve_part(blk, x, o, h0, HALF, ACT_ODD.get((blk, h), 0))
                    if blk < 6:
                        # Store gate: a no-op rewrite of the last output
                        # column that reads the last x tile. This gives the
                        # store a data dependency on the final load, so the
                        # dataflow scheduler cannot interleave store DMAs
                        # ahead of the last loads (which starves the PE of
                        # x7 and stretches the load phase).
                        e = h0 + HALF - 1
                        nc.vector.scalar_tensor_tensor(
                            o[:, e : e + 1], xts[NBLK - 1][:, 0:1], 0.0,
                            o[:, e : e + 1], mult, add,
                        )
                    nc.sync.dma_start(
                        out=ot[blk * 128 : (blk + 1) * 128, h0 : h0 + HALF],
                        in_=o[:, h0 : h0 + HALF],
                    )
                outs.append(o)
    nc.compile()
    return nc


def _shard_inputs(x, w):
    in_maps = []
    for core in range(N_CORES):
        b, half = divmod(core, 2)
        d0 = half * DH
        xt = np.zeros((DH, ROWW), dtype=np.float16)
        xt[:, PAD : PAD + L] = x[b, :, d0 : d0 + DH].T.astype(np.float16)
        # w rows for this shard, rearranged so partition p holds the K
        # weights of channel blk*128 + p at free cols [blk*K, blk*K + K)
        w_sh = w[d0 : d0 + DH].reshape(NBLK, 128, K)
        wt = (
            w_sh.transpose(1, 0, 2).reshape(128, NBLK * K).astype(np.float32)
        )
        # diag(w_j) slabs for the PE blocks: partition p, col j*128 + m
        # holds w[blk*128+p, j] iff m == p else 0
        wdv = np.zeros((128, len(PE_DIAG_BLKS), K, 128), dtype=np.float16)
        idx = np.arange(128)
        for i, blk in enumerate(PE_DIAG_BLKS):
            wdv[idx, i, :, idx] = w_sh[blk].astype(np.float16)
        in_maps.append(
            {
                "xt": np.ascontiguousarray(xt),
                "wt": np.ascontiguousarray(wt),
                "wd": np.ascontiguousarray(
                    wdv.reshape(128, len(PE_DIAG_BLKS) * K * 128)
                ),
            }
        )
    return in_maps


def kernel(x, w):
    x = np.asarray(x, dtype=np.float32)
    w = np.asarray(w, dtype=np.float32)
    assert x.shape == (B, L, D) and w.shape == (D, K)

    if "nc" not in _cache:
        _cache["nc"] = _build_bass()
    nc = _cache["nc"]

    in_maps = _shard_inputs(x, w)
    res = None
    for attempt in range(3):
        try:
            res = run_bass_kernel_spmd(nc, in_maps, core_ids=list(range(N_CORES)))
            break
        except Exception:
            if attempt == 2:
                raise
    _cache["last_results"] = res

    out = np.empty((B, L, D), dtype=np.float32)
    for core in range(N_CORES):
        b, half = divmod(core, 2)
        d0 = half * DH
        out[b, :, d0 : d0 + DH] = res.results[core]["ot"].T.astype(np.float32)
    return out


# revision 22
# speedup vs baseline: 1.0065x; 1.0065x over previous
"""Depthwise causal Conv1d (k=4) + SiLU on 8 Trainium2 NeuronCores.

Problem: x [4, 4096, 2048] f32, w [2048, 4] f32,
out[b, t, d] = silu(sum_j w[d, j] * x[b, t - 3 + j, d])   (zero-padded left).

Sharding: 8 cores = 4 batches x 2 channel-halves. Depthwise conv is
independent per channel, so channel sharding needs no halo exchange.

Layout: each core receives its shard host-transposed to [channels, time]
(channels on SBUF partitions). The per-channel weight w[d, j] is a
per-partition scalar and the causal time shifts are free-dim AP offsets
into one loaded tile.

The kernel is HBM-bandwidth-bound (~16.8 MB of fp16 I/O per core against
~390 GB/s effective per-NeuronCore HBM), with compute only just fitting
under the DMA window, so the schedule works both angles:
 - All 8 input-block loads are queued back to back at the head of the
   sync HWDGE ring; stores are issued on the SAME ring afterwards. The
   ring drains FIFO, which gives loads strict priority over stores
   (loads finish at ~30 us) and keeps the SDMA engines fed to the end.
 - Compute is spread over FOUR engine lanes so no engine exceeds the
   DMA window: odd blocks run on the TensorEngine as diag(w_j) matmuls
   accumulating the 4 taps in PSUM (diag built on-chip from a 32 KB
   host-sent identity); even blocks run elementwise with the products
   split between DVE and GpSimd (GpSimd is otherwise idle since stores
   left the SWDGE path) and the pair-add tree on DVE. ACT does only
   SiLU.

Measured op rates (fp16): DVE tensor_scalar 0.29 ns/elem, DVE
tensor_tensor 0.54, GpSimd tensor_scalar ~0.9, ACT ~1.0, PE ~9.3 us per
4-tap 4096-col block.

Precision: x and the output are host-cast fp16 (halves HBM traffic both
ways); products and adds stay fp16 (PE accumulates fp32 in PSUM); SiLU
computes fp32-internally on ACT. End-to-end relative error ~5e-4.
"""

import sys
import types

import numpy as np

import concourse.bass as bass
import concourse.bacc as bacc
import concourse.mybir as mybir
from concourse.tile import TileContext
from concourse.bass_utils import run_bass_kernel_spmd


def _ensure_ntff_hook():
    """bass_utils imports antenv.axon_hooks when BASS_TRACE is set; that
    module is absent on this image. Install a shim so tracing works when
    possible and degrades gracefully (instead of crashing) when not."""
    try:
        import antenv.axon_hooks  # noqa: F401

        return
    except ImportError:
        pass
    try:
        import antenv

        hook = None
        try:
            if "/root/.axon_site" not in sys.path:
                sys.path.insert(0, "/root/.axon_site")
            from trn_agent_boot.trn_boot import _ntff_profile_via_ctypes

            hook = _ntff_profile_via_ctypes("/opt/axon/libaxon_pjrt.so")
        except Exception:
            hook = None
        mod = types.ModuleType("antenv.axon_hooks")
        mod._hook = hook
        mod.get_axon_ntff_profile_hook = lambda: mod._hook
        mod.set_axon_ntff_profile_hook = lambda h: setattr(mod, "_hook", h)
        sys.modules["antenv.axon_hooks"] = mod
        antenv.axon_hooks = mod
    except Exception:
        pass


_ensure_ntff_hook()

B, L, D = 4, 4096, 2048
K = 4
PAD = K - 1
N_CORES = 8
DH = D // 2            # channels per core
NBLK = DH // 128       # 128-partition channel blocks per core
ROWW = 4128            # DRAM row stride (fp16 elems): 64B-aligned rows

MID_DT = mybir.dt.float16
PE_BLKS = (1, 3, 5, 7)  # blocks computed fully on the TensorEngine
PE_HALF_BLK = 6         # block 6: first 2048 cols on DVE, rest on the PE
PE_DIAG_BLKS = (1, 3, 5, 6, 7)  # DRAM order of per-block diag(w_j) slabs

_cache = {}


def _build_bass():
    nc = bacc.Bacc()
    xt = nc.dram_tensor("xt", [DH, ROWW], MID_DT, kind="ExternalInput")
    wt = nc.dram_tensor("wt", [128, NBLK * K], mybir.dt.float32, kind="ExternalInput")
    # host-built diag(w_j) blocks for the PE path, one [K*128] slab per
    # PE block in PE_DIAG_BLKS order
    wd = nc.dram_tensor(
        "wd", [128, len(PE_DIAG_BLKS) * K * 128], MID_DT, kind="ExternalInput"
    )
    ot = nc.dram_tensor("ot", [DH, L], MID_DT, kind="ExternalOutput")
    f32 = mybir.dt.float32
    HALF = L // 2

    with TileContext(nc) as tc:
        with tc.tile_pool(name="pool", bufs=2) as pool, \
             tc.tile_pool(name="psum", bufs=2, space="PSUM") as psum_pool:
            # w leads the sync ring so compute can start as soon as the
            # first x block lands.
            w = pool.tile([128, NBLK * K], f32, tag="w", bufs=1)
            nc.sync.dma_start(out=w[:], in_=wt[:, :])
            # Warmup: a tiny Silu forces the silu activation-table set to
            # load during the initial DMA wait; it is the only table load
            # in the whole kernel.
            warm = pool.tile([128, 2], MID_DT, tag="warm", bufs=1)
            nc.vector.memset(warm[:], 0.0)
            nc.scalar.activation(warm[:], warm[:], mybir.ActivationFunctionType.Silu)

            # All loads up front, back to back on the sync ring. The diag
            # slab for each PE block is interleaved right after that
            # block's x so it lands just in time (DMA has slack; DVE/ACT
            # do not, so the diag is host-built rather than built on-chip).
            # x7 loads before x6 so the PE (which owns all of block 7 but
            # only half of block 6) is fed in its processing order.
            LOAD_ORDER = [0, 1, 2, 3, 4, 5, 7, 6]
            xts = {}
            wdts = {}
            for blk in LOAD_ORDER:
                x = pool.tile([128, L + PAD + 1], MID_DT, tag="x", bufs=NBLK)
                nc.sync.dma_start(
                    out=x[:, 0 : L + PAD],
                    in_=xt[blk * 128 : (blk + 1) * 128, 0 : L + PAD],
                )
                xts[blk] = x
                if blk in PE_DIAG_BLKS:
                    wslab = pool.tile(
                        [128, K * 128], MID_DT, tag="wd", bufs=len(PE_DIAG_BLKS)
                    )
                    c = PE_DIAG_BLKS.index(blk) * K * 128
                    nc.sync.dma_start(out=wslab[:], in_=wd[:, c : c + K * 128])
                    wdts[blk] = wslab
            x_last = xts[LOAD_ORDER[-1]]

            PQ = 1024

            def pe_part(blk, x, o, t0, size):
                # TensorEngine path for [t0, t0+size): accumulate the 4
                # diag(w_j) matmuls per 512-col PSUM chunk (shift =
                # free-dim offset on the moving operand), SiLU from PSUM.
                # 1024-col PSUM quarters, 4 deep, so the PE stays 3 ahead
                # of ACT's PSUM drain.
                for q0 in range(t0, t0 + size, PQ):
                    ps = psum_pool.tile([128, PQ], f32, tag="ps", bufs=4)
                    for j in range(K):
                        lw = wdts[blk][:, j * 128 : (j + 1) * 128]
                        for cc in range(PQ // 512):
                            nc.tensor.matmul(
                                ps[:, cc * 512 : (cc + 1) * 512],
                                lw,
                                x[:, q0 + cc * 512 + j : q0 + cc * 512 + j + 512],
                                start=(j == 0),
                                stop=(j == K - 1),
                            )
                    nc.scalar.activation(
                        o[:, q0 : q0 + PQ], ps[:], mybir.ActivationFunctionType.Silu
                    )

            def dve_part(blk, x, o, t0, size, n_act=0):
                # Elementwise path for [t0, t0+size): qe holds the
                # even-shift products [q0 | q2], qo the odd [q1 | q3]
                # (n_act of the odd ones on ACT to relieve DVE), pair-add
                # + final add on DVE, then SiLU. Shift-rebased:
                # q_j[:, t] = w_j * x[:, t + j].
                wj = lambda j: w[:, blk * K + j : blk * K + j + 1]
                qe = pool.tile([128, 2, size], MID_DT, tag=f"qe{size}", bufs=3)
                qo = pool.tile([128, 2, size], MID_DT, tag=f"qo{size}", bufs=3)
                if n_act >= 1:
                    nc.scalar.mul(qo[:, 0, :], x[:, t0 + 1 : t0 + 1 + size], wj(1))
                else:
                    nc.vector.tensor_scalar_mul(
                        qo[:, 0, :], x[:, t0 + 1 : t0 + 1 + size], wj(1)
                    )
                nc.vector.tensor_scalar_mul(qe[:, 0, :], x[:, t0 : t0 + size], wj(0))
                if n_act >= 2:
                    nc.scalar.mul(qo[:, 1, :], x[:, t0 + 3 : t0 + 3 + size], wj(3))
                else:
                    nc.vector.tensor_scalar_mul(
                        qo[:, 1, :], x[:, t0 + 3 : t0 + 3 + size], wj(3)
                    )
                nc.vector.tensor_scalar_mul(qe[:, 1, :], x[:, t0 + 2 : t0 + 2 + size], wj(2))
                nc.vector.tensor_add(qe[:, :, :], qe[:, :, :], qo[:, :, :])
                nc.vector.tensor_add(qe[:, 0, :], qe[:, 0, :], qe[:, 1, :])
                nc.scalar.activation(
                    o[:, t0 : t0 + size], qe[:, 0, :],
                    mybir.ActivationFunctionType.Silu,
                )

            # odd products on ACT per (blk, half): balances DVE ~40 / ACT ~40
            ACT_ODD = {(0, 0): 2, (0, 1): 1}
            mult = mybir.AluOpType.mult
            add = mybir.AluOpType.add
            # Block 7 is processed before block 6, and block 6's PE half
            # before its DVE half, so ACT's (statically ordered) SiLU queue
            # ends with exactly the last-ready work — otherwise the PE's
            # final PSUM quarters head-of-line block behind a late DVE SiLU.
            for blk in [0, 1, 2, 3, 4, 5, 7, 6]:
                x = xts[blk]
                o = pool.tile([128, L], MID_DT, tag="o", bufs=NBLK)
                halves = [1, 0] if blk == PE_HALF_BLK else [0, 1]
                for h in halves:
                    h0 = h * HALF
                    if blk in PE_BLKS or (blk == PE_HALF_BLK and h == 1):
                        pe_part(blk, x, o, h0, HALF)
                    elif blk == PE_HALF_BLK:
                        # the tail of the whole kernel: 1024-col pieces so
                        # the final SiLU+store chain is short
                        dve_part(blk, x, o, h0, PQ)
                        dve_part(blk, x, o, h0 + PQ, PQ)
                    else:
                        dve_part(blk, x, o, h0, HALF, ACT_ODD.get((blk, h), 0))
                    if blk != PE_HALF_BLK:
                        # Store gate: a no-op rewrite of the last output
                        # column that reads the last-loaded x tile. This
                        # gives the store a data dependency on the final
                        # load, so the dataflow scheduler cannot interleave
                        # store DMAs ahead of the last loads (which starves
                        # the PE and stretches the load phase).
                        e = h0 + HALF - 1
                        nc.vector.scalar_tensor_tensor(
                            o[:, e : e + 1], x_last[:, 0:1], 0.0,
                            o[:, e : e + 1], mult, add,
                        )
                    nc.sync.dma_start(
                        out=ot[blk * 128 : (blk + 1) * 128, h0 : h0 + HALF],
                        in_=o[:, h0 : h0 + HALF],
                    )
    nc.compile()
    return nc


def _shard_inputs(x, w):
    in_maps = []
    for core in range(N_CORES):
        b, half = divmod(core, 2)
        d0 = half * DH
        xt = np.zeros((DH, ROWW), dtype=np.float16)
        xt[:, PAD : PAD + L] = x[b, :, d0 : d0 + DH].T.astype(np.float16)
        # w rows for this shard, rearranged so partition p holds the K
        # weights of channel blk*128 + p at free cols [blk*K, blk*K + K)
        w_sh = w[d0 : d0 + DH].reshape(NBLK, 128, K)
        wt = (
            w_sh.transpose(1, 0, 2).reshape(128, NBLK * K).astype(np.float32)
        )
        # diag(w_j) slabs for the PE blocks: partition p, col j*128 + m
        # holds w[blk*128+p, j] iff m == p else 0
        wdv = np.zeros((128, len(PE_DIAG_BLKS), K, 128), dtype=np.float16)
        idx = np.arange(128)
        for i, blk in enumerate(PE_DIAG_BLKS):
            wdv[idx, i, :, idx] = w_sh[blk].astype(np.float16)
        in_maps.append(
            {
                "xt": np.ascontiguousarray(xt),
                "wt": np.ascontiguousarray(wt),
                "wd": np.ascontiguousarray(
                    wdv.reshape(128, len(PE_DIAG_BLKS) * K * 128)
                ),
            }
        )
    return in_maps


def kernel(x, w):
    x = np.asarray(x, dtype=np.float32)
    w = np.asarray(w, dtype=np.float32)
    assert x.shape == (B, L, D) and w.shape == (D, K)

    if "nc" not in _cache:
        _cache["nc"] = _build_bass()
    nc = _cache["nc"]

    in_maps = _shard_inputs(x, w)
    res = None
    for attempt in range(3):
        try:
            res = run_bass_kernel_spmd(nc, in_maps, core_ids=list(range(N_CORES)))
            break
        except Exception:
            if attempt == 2:
                raise
    _cache["last_results"] = res

    out = np.empty((B, L, D), dtype=np.float32)
    for core in range(N_CORES):
        b, half = divmod(core, 2)
        d0 = half * DH
        out[b, :, d0 : d0 + DH] = res.results[core]["ot"].T.astype(np.float32)
    return out


# revision 25
# speedup vs baseline: 1.0139x; 1.0073x over previous
"""Depthwise causal Conv1d (k=4) + SiLU on 8 Trainium2 NeuronCores.

Problem: x [4, 4096, 2048] f32, w [2048, 4] f32,
out[b, t, d] = silu(sum_j w[d, j] * x[b, t - 3 + j, d])   (zero-padded left).

Sharding: 8 cores = 4 batches x 2 channel-halves. Depthwise conv is
independent per channel, so channel sharding needs no halo exchange.

Layout: each core receives its shard host-transposed to [channels, time]
(channels on SBUF partitions). The per-channel weight w[d, j] is a
per-partition scalar and the causal time shifts are free-dim AP offsets
into one loaded tile.

The kernel is HBM-bandwidth-bound (~16.8 MB of fp16 I/O per core against
~390 GB/s effective per-NeuronCore HBM), with compute only just fitting
under the DMA window, so the schedule works both angles:
 - All 8 input-block loads are queued back to back at the head of the
   sync HWDGE ring; stores are issued on the SAME ring afterwards. The
   ring drains FIFO, which gives loads strict priority over stores
   (loads finish at ~30 us) and keeps the SDMA engines fed to the end.
 - Compute is spread over FOUR engine lanes so no engine exceeds the
   DMA window: odd blocks run on the TensorEngine as diag(w_j) matmuls
   accumulating the 4 taps in PSUM (diag built on-chip from a 32 KB
   host-sent identity); even blocks run elementwise with the products
   split between DVE and GpSimd (GpSimd is otherwise idle since stores
   left the SWDGE path) and the pair-add tree on DVE. ACT does only
   SiLU.

Measured op rates (fp16): DVE tensor_scalar 0.29 ns/elem, DVE
tensor_tensor 0.54, GpSimd tensor_scalar ~0.9, ACT ~1.0, PE ~9.3 us per
4-tap 4096-col block.

Precision: x and the output are host-cast fp16 (halves HBM traffic both
ways); products and adds stay fp16 (PE accumulates fp32 in PSUM); SiLU
computes fp32-internally on ACT. End-to-end relative error ~5e-4.
"""

import sys
import types

import numpy as np

import concourse.bass as bass
import concourse.bacc as bacc
import concourse.mybir as mybir
from concourse.tile import TileContext
from concourse.bass_utils import run_bass_kernel_spmd


def _ensure_ntff_hook():
    """bass_utils imports antenv.axon_hooks when BASS_TRACE is set; that
    module is absent on this image. Install a shim so tracing works when
    possible and degrades gracefully (instead of crashing) when not."""
    try:
        import antenv.axon_hooks  # noqa: F401

        return
    except ImportError:
        pass
    try:
        import antenv

        hook = None
        try:
            if "/root/.axon_site" not in sys.path:
                sys.path.insert(0, "/root/.axon_site")
            from trn_agent_boot.trn_boot import _ntff_profile_via_ctypes

            hook = _ntff_profile_via_ctypes("/opt/axon/libaxon_pjrt.so")
        except Exception:
            hook = None
        mod = types.ModuleType("antenv.axon_hooks")
        mod._hook = hook
        mod.get_axon_ntff_profile_hook = lambda: mod._hook
        mod.set_axon_ntff_profile_hook = lambda h: setattr(mod, "_hook", h)
        sys.modules["antenv.axon_hooks"] = mod
        antenv.axon_hooks = mod
    except Exception:
        pass


_ensure_ntff_hook()

B, L, D = 4, 4096, 2048
K = 4
PAD = K - 1
N_CORES = 8
DH = D // 2            # channels per core
NBLK = DH // 128       # 128-partition channel blocks per core
ROWW = 4128            # DRAM row stride (fp16 elems): 64B-aligned rows

MID_DT = mybir.dt.float16
PE_BLKS = (1, 3, 5, 7)  # blocks computed fully on the TensorEngine
PE_HALF_BLK = 6         # block 6: first 2048 cols on DVE, rest on the PE
PE_DIAG_BLKS = (1, 3, 5, 6, 7)  # DRAM order of per-block diag(w_j) slabs

_cache = {}


def _build_bass():
    nc = bacc.Bacc()
    xt = nc.dram_tensor("xt", [DH, ROWW], MID_DT, kind="ExternalInput")
    wt = nc.dram_tensor("wt", [128, NBLK * K], mybir.dt.float32, kind="ExternalInput")
    # host-built diag(w_j) blocks for the PE path, one [K*128] slab per
    # PE block in PE_DIAG_BLKS order
    wd = nc.dram_tensor(
        "wd", [128, len(PE_DIAG_BLKS) * K * 128], MID_DT, kind="ExternalInput"
    )
    ot = nc.dram_tensor("ot", [DH, L], MID_DT, kind="ExternalOutput")
    f32 = mybir.dt.float32
    HALF = L // 2

    with TileContext(nc) as tc:
        with tc.tile_pool(name="pool", bufs=2) as pool, \
             tc.tile_pool(name="psum", bufs=2, space="PSUM") as psum_pool:
            # w leads the sync ring so compute can start as soon as the
            # first x block lands.
            w = pool.tile([128, NBLK * K], f32, tag="w", bufs=1)
            nc.sync.dma_start(out=w[:], in_=wt[:, :])
            # Warmup: a tiny Silu forces the silu activation-table set to
            # load during the initial DMA wait; it is the only table load
            # in the whole kernel.
            warm = pool.tile([128, 2], MID_DT, tag="warm", bufs=1)
            nc.vector.memset(warm[:], 0.0)
            nc.scalar.activation(warm[:], warm[:], mybir.ActivationFunctionType.Silu)

            # All loads up front, back to back on the sync ring. The diag
            # slab for each PE block is interleaved right after that
            # block's x so it lands just in time (DMA has slack; DVE/ACT
            # do not, so the diag is host-built rather than built on-chip).
            # x7 loads before x6 so the PE (which owns all of block 7 but
            # only half of block 6) is fed in its processing order.
            LOAD_ORDER = [0, 1, 2, 3, 4, 5, 7, 6]
            # The first two blocks load in pieces so compute starts ~3 us
            # sooner (the piece lands in ~0.7 us instead of 2.7); later
            # blocks load whole — the pipeline is already full by then.
            N_PIECES = {0: 4, 1: 2}
            xts = {}
            wdts = {}
            for blk in LOAD_ORDER:
                x = pool.tile([128, L + PAD + 1], MID_DT, tag="x", bufs=NBLK)
                n_p = N_PIECES.get(blk, 1)
                step = L // n_p
                # non-overlapping pieces (first one carries the PAD) so a
                # compute chunk only depends on the pieces it actually reads
                cuts = [0] + [p * step + PAD for p in range(1, n_p)] + [L + PAD]
                for t0, t1 in zip(cuts[:-1], cuts[1:]):
                    nc.sync.dma_start(
                        out=x[:, t0:t1],
                        in_=xt[blk * 128 : (blk + 1) * 128, t0:t1],
                    )
                xts[blk] = x
                if blk in PE_DIAG_BLKS:
                    wslab = pool.tile(
                        [128, K * 128], MID_DT, tag="wd", bufs=len(PE_DIAG_BLKS)
                    )
                    c = PE_DIAG_BLKS.index(blk) * K * 128
                    nc.sync.dma_start(out=wslab[:], in_=wd[:, c : c + K * 128])
                    wdts[blk] = wslab
            x_last = xts[LOAD_ORDER[-1]]

            PQ = 1024

            def pe_part(blk, x, o, t0, size):
                # TensorEngine path for [t0, t0+size): accumulate the 4
                # diag(w_j) matmuls per 512-col PSUM chunk (shift =
                # free-dim offset on the moving operand), SiLU from PSUM.
                # 1024-col PSUM quarters, 4 deep, so the PE stays 3 ahead
                # of ACT's PSUM drain.
                for q0 in range(t0, t0 + size, PQ):
                    ps = psum_pool.tile([128, PQ], f32, tag="ps", bufs=4)
                    for j in range(K):
                        lw = wdts[blk][:, j * 128 : (j + 1) * 128]
                        for cc in range(PQ // 512):
                            nc.tensor.matmul(
                                ps[:, cc * 512 : (cc + 1) * 512],
                                lw,
                                x[:, q0 + cc * 512 + j : q0 + cc * 512 + j + 512],
                                start=(j == 0),
                                stop=(j == K - 1),
                            )
                    nc.scalar.activation(
                        o[:, q0 : q0 + PQ], ps[:], mybir.ActivationFunctionType.Silu
                    )

            def dve_part(blk, x, o, t0, size, n_act=0):
                # Elementwise path for [t0, t0+size): qe holds the
                # even-shift products [q0 | q2], qo the odd [q1 | q3]
                # (n_act of the odd ones on ACT to relieve DVE), pair-add
                # + final add on DVE, then SiLU. Shift-rebased:
                # q_j[:, t] = w_j * x[:, t + j].
                wj = lambda j: w[:, blk * K + j : blk * K + j + 1]
                qe = pool.tile([128, 2, size], MID_DT, tag=f"qe{size}", bufs=3)
                qo = pool.tile([128, 2, size], MID_DT, tag=f"qo{size}", bufs=3)
                if n_act >= 1:
                    nc.scalar.mul(qo[:, 0, :], x[:, t0 + 1 : t0 + 1 + size], wj(1))
                else:
                    nc.vector.tensor_scalar_mul(
                        qo[:, 0, :], x[:, t0 + 1 : t0 + 1 + size], wj(1)
                    )
                nc.vector.tensor_scalar_mul(qe[:, 0, :], x[:, t0 : t0 + size], wj(0))
                if n_act >= 2:
                    nc.scalar.mul(qo[:, 1, :], x[:, t0 + 3 : t0 + 3 + size], wj(3))
                else:
                    nc.vector.tensor_scalar_mul(
                        qo[:, 1, :], x[:, t0 + 3 : t0 + 3 + size], wj(3)
                    )
                nc.vector.tensor_scalar_mul(qe[:, 1, :], x[:, t0 + 2 : t0 + 2 + size], wj(2))
                nc.vector.tensor_add(qe[:, :, :], qe[:, :, :], qo[:, :, :])
                nc.vector.tensor_add(qe[:, 0, :], qe[:, 0, :], qe[:, 1, :])
                nc.scalar.activation(
                    o[:, t0 : t0 + size], qe[:, 0, :],
                    mybir.ActivationFunctionType.Silu,
                )

            # odd products on ACT per (blk, half): balances DVE ~40 / ACT ~40
            ACT_ODD = {(0, 0): 2, (0, 1): 1}
            mult = mybir.AluOpType.mult
            add = mybir.AluOpType.add
            # Block 7 is processed before block 6, and block 6's PE half
            # before its DVE half, so ACT's (statically ordered) SiLU queue
            # ends with exactly the last-ready work — otherwise the PE's
            # final PSUM quarters head-of-line block behind a late DVE SiLU.
            for blk in [0, 1, 2, 3, 4, 5, 7, 6]:
                x = xts[blk]
                o = pool.tile([128, L], MID_DT, tag="o", bufs=NBLK)
                halves = [1, 0] if blk == PE_HALF_BLK else [0, 1]
                for h in halves:
                    h0 = h * HALF
                    if blk in PE_BLKS or (blk == PE_HALF_BLK and h == 1):
                        pe_part(blk, x, o, h0, HALF)
                    elif blk == PE_HALF_BLK or blk == 0:
                        # 1024-col pieces: block 0 so compute ramps with
                        # the piecewise first load, block 6 so the final
                        # SiLU+store chain is short
                        n_act = ACT_ODD.get((blk, h), 0)
                        dve_part(blk, x, o, h0, PQ, n_act)
                        dve_part(blk, x, o, h0 + PQ, PQ, n_act)
                    else:
                        dve_part(blk, x, o, h0, HALF, ACT_ODD.get((blk, h), 0))
                    if blk != PE_HALF_BLK:
                        # Store gate: a no-op rewrite of the last output
                        # column that reads the last-loaded x tile. This
                        # gives the store a data dependency on the final
                        # load, so the dataflow scheduler cannot interleave
                        # store DMAs ahead of the last loads (which starves
                        # the PE and stretches the load phase).
                        e = h0 + HALF - 1
                        nc.vector.scalar_tensor_tensor(
                            o[:, e : e + 1], x_last[:, 0:1], 0.0,
                            o[:, e : e + 1], mult, add,
                        )
                    nc.sync.dma_start(
                        out=ot[blk * 128 : (blk + 1) * 128, h0 : h0 + HALF],
                        in_=o[:, h0 : h0 + HALF],
                    )
    nc.compile()
    return nc


def _shard_inputs(x, w):
    in_maps = []
    for core in range(N_CORES):
        b, half = divmod(core, 2)
        d0 = half * DH
        xt = np.zeros((DH, ROWW), dtype=np.float16)
        xt[:, PAD : PAD + L] = x[b, :, d0 : d0 + DH].T.astype(np.float16)
        # w rows for this shard, rearranged so partition p holds the K
        # weights of channel blk*128 + p at free cols [blk*K, blk*K + K)
        w_sh = w[d0 : d0 + DH].reshape(NBLK, 128, K)
        wt = (
            w_sh.transpose(1, 0, 2).reshape(128, NBLK * K).astype(np.float32)
        )
        # diag(w_j) slabs for the PE blocks: partition p, col j*128 + m
        # holds w[blk*128+p, j] iff m == p else 0
        wdv = np.zeros((128, len(PE_DIAG_BLKS), K, 128), dtype=np.float16)
        idx = np.arange(128)
        for i, blk in enumerate(PE_DIAG_BLKS):
            wdv[idx, i, :, idx] = w_sh[blk].astype(np.float16)
        in_maps.append(
            {
                "xt": np.ascontiguousarray(xt),
                "wt": np.ascontiguousarray(wt),
                "wd": np.ascontiguousarray(
                    wdv.reshape(128, len(PE_DIAG_BLKS) * K * 128)
                ),
            }
        )
    return in_maps


def kernel(x, w):
    x = np.asarray(x, dtype=np.float32)
    w = np.asarray(w, dtype=np.float32)
    assert x.shape == (B, L, D) and w.shape == (D, K)

    if "nc" not in _cache:
        _cache["nc"] = _build_bass()
    nc = _cache["nc"]

    in_maps = _shard_inputs(x, w)
    res = None
    for attempt in range(3):
        try:
            res = run_bass_kernel_spmd(nc, in_maps, core_ids=list(range(N_CORES)))
            break
        except Exception:
            if attempt == 2:
                raise
    _cache["last_results"] = res

    out = np.empty((B, L, D), dtype=np.float32)
    for core in range(N_CORES):
        b, half = divmod(core, 2)
        d0 = half * DH
        out[b, :, d0 : d0 + DH] = res.results[core]["ot"].T.astype(np.float32)
    return out


# revision 27
# speedup vs baseline: 1.0285x; 1.0144x over previous
"""Depthwise causal Conv1d (k=4) + SiLU on 8 Trainium2 NeuronCores.

Problem: x [4, 4096, 2048] f32, w [2048, 4] f32,
out[b, t, d] = silu(sum_j w[d, j] * x[b, t - 3 + j, d])   (zero-padded left).

Sharding: 8 cores = 4 batches x 2 channel-halves. Depthwise conv is
independent per channel, so channel sharding needs no halo exchange.

Layout: each core receives its shard host-transposed to [channels, time]
(channels on SBUF partitions). The per-channel weight w[d, j] is a
per-partition scalar and the causal time shifts are free-dim AP offsets
into one loaded tile.

The kernel is HBM-bandwidth-bound (~16.8 MB of fp16 I/O per core against
~390 GB/s effective per-NeuronCore HBM), with compute only just fitting
under the DMA window, so the schedule works both angles:
 - All 8 input-block loads are queued back to back at the head of the
   sync HWDGE ring; stores are issued on the SAME ring afterwards. The
   ring drains FIFO, which gives loads strict priority over stores
   (loads finish at ~30 us) and keeps the SDMA engines fed to the end.
 - Compute is spread over FOUR engine lanes so no engine exceeds the
   DMA window: odd blocks run on the TensorEngine as diag(w_j) matmuls
   accumulating the 4 taps in PSUM (diag built on-chip from a 32 KB
   host-sent identity); even blocks run elementwise with the products
   split between DVE and GpSimd (GpSimd is otherwise idle since stores
   left the SWDGE path) and the pair-add tree on DVE. ACT does only
   SiLU.

Measured op rates (fp16): DVE tensor_scalar 0.29 ns/elem, DVE
tensor_tensor 0.54, GpSimd tensor_scalar ~0.9, ACT ~1.0, PE ~9.3 us per
4-tap 4096-col block.

Precision: x and the output are host-cast fp16 (halves HBM traffic both
ways); products and adds stay fp16 (PE accumulates fp32 in PSUM); SiLU
computes fp32-internally on ACT. End-to-end relative error ~5e-4.
"""

import sys
import types

import numpy as np

import concourse.bass as bass
import concourse.bacc as bacc
import concourse.mybir as mybir
from concourse.tile import TileContext
from concourse.bass_utils import run_bass_kernel_spmd


def _ensure_ntff_hook():
    """bass_utils imports antenv.axon_hooks when BASS_TRACE is set; that
    module is absent on this image. Install a shim so tracing works when
    possible and degrades gracefully (instead of crashing) when not."""
    try:
        import antenv.axon_hooks  # noqa: F401

        return
    except ImportError:
        pass
    try:
        import antenv

        hook = None
        try:
            if "/root/.axon_site" not in sys.path:
                sys.path.insert(0, "/root/.axon_site")
            from trn_agent_boot.trn_boot import _ntff_profile_via_ctypes

            hook = _ntff_profile_via_ctypes("/opt/axon/libaxon_pjrt.so")
        except Exception:
            hook = None
        mod = types.ModuleType("antenv.axon_hooks")
        mod._hook = hook
        mod.get_axon_ntff_profile_hook = lambda: mod._hook
        mod.set_axon_ntff_profile_hook = lambda h: setattr(mod, "_hook", h)
        sys.modules["antenv.axon_hooks"] = mod
        antenv.axon_hooks = mod
    except Exception:
        pass


_ensure_ntff_hook()

B, L, D = 4, 4096, 2048
K = 4
PAD = K - 1
N_CORES = 8
DH = D // 2            # channels per core
NBLK = DH // 128       # 128-partition channel blocks per core
ROWW = 4128            # DRAM row stride (fp16 elems): 64B-aligned rows

MID_DT = mybir.dt.float16
PE_BLKS = (1, 3, 5, 7)  # blocks computed fully on the TensorEngine
PE_HALF_BLK = 6         # block 6: first 2048 cols on DVE, rest on the PE
PE_DIAG_BLKS = (1, 3, 5, 6, 7)  # DRAM order of per-block diag(w_j) slabs

_cache = {}


def _build_bass():
    nc = bacc.Bacc()
    xt = nc.dram_tensor("xt", [DH, ROWW], MID_DT, kind="ExternalInput")
    wt = nc.dram_tensor("wt", [128, NBLK * K], mybir.dt.float32, kind="ExternalInput")
    # host-built diag(w_j) blocks for the PE path, one [K*128] slab per
    # PE block in PE_DIAG_BLKS order
    wd = nc.dram_tensor(
        "wd", [128, len(PE_DIAG_BLKS) * K * 128], MID_DT, kind="ExternalInput"
    )
    ot = nc.dram_tensor("ot", [DH, L], MID_DT, kind="ExternalOutput")
    f32 = mybir.dt.float32
    HALF = L // 2

    with TileContext(nc) as tc:
        with tc.tile_pool(name="pool", bufs=2) as pool, \
             tc.tile_pool(name="psum", bufs=2, space="PSUM") as psum_pool:
            # w leads the sync ring so compute can start as soon as the
            # first x block lands.
            w = pool.tile([128, NBLK * K], f32, tag="w", bufs=1)
            nc.sync.dma_start(out=w[:], in_=wt[:, :])
            # Warmup: a tiny Silu forces the silu activation-table set to
            # load during the initial DMA wait; it is the only table load
            # in the whole kernel.
            warm = pool.tile([128, 2], MID_DT, tag="warm", bufs=1)
            nc.vector.memset(warm[:], 0.0)
            nc.scalar.activation(warm[:], warm[:], mybir.ActivationFunctionType.Silu)

            # All loads up front, back to back on the sync ring. The diag
            # slab for each PE block is interleaved right after that
            # block's x so it lands just in time (DMA has slack; DVE/ACT
            # do not, so the diag is host-built rather than built on-chip).
            # x7 loads before x6 so the PE (which owns all of block 7 but
            # only half of block 6) is fed in its processing order.
            LOAD_ORDER = [0, 1, 2, 3, 4, 5, 7, 6]
            # The first two blocks load in pieces so compute starts ~3 us
            # sooner (the piece lands in ~0.7 us instead of 2.7); later
            # blocks load whole — the pipeline is already full by then.
            N_PIECES = {0: 4, 1: 2}
            xts = {}
            wdts = {}
            for blk in LOAD_ORDER:
                if blk in PE_DIAG_BLKS:
                    # diag slab BEFORE the x pieces: the PE needs it first
                    wslab = pool.tile(
                        [128, K * 128], MID_DT, tag="wd", bufs=len(PE_DIAG_BLKS)
                    )
                    c = PE_DIAG_BLKS.index(blk) * K * 128
                    nc.sync.dma_start(out=wslab[:], in_=wd[:, c : c + K * 128])
                    wdts[blk] = wslab
                x = pool.tile([128, L + PAD + 1], MID_DT, tag="x", bufs=NBLK)
                n_p = N_PIECES.get(blk, 1)
                step = L // n_p
                # non-overlapping pieces (first one carries the PAD) so a
                # compute chunk only depends on the pieces it actually reads
                cuts = [0] + [p * step + PAD for p in range(1, n_p)] + [L + PAD]
                for t0, t1 in zip(cuts[:-1], cuts[1:]):
                    nc.sync.dma_start(
                        out=x[:, t0:t1],
                        in_=xt[blk * 128 : (blk + 1) * 128, t0:t1],
                    )
                xts[blk] = x
            x_last = xts[LOAD_ORDER[-1]]

            PQ = 1024

            def pe_part(blk, x, o, t0, size):
                # TensorEngine path for [t0, t0+size): accumulate the 4
                # diag(w_j) matmuls per 512-col PSUM chunk (shift =
                # free-dim offset on the moving operand), SiLU from PSUM.
                # 1024-col PSUM quarters, 4 deep, so the PE stays 3 ahead
                # of ACT's PSUM drain.
                for q0 in range(t0, t0 + size, PQ):
                    ps = psum_pool.tile([128, PQ], f32, tag="ps", bufs=4)
                    for j in range(K):
                        lw = wdts[blk][:, j * 128 : (j + 1) * 128]
                        for cc in range(PQ // 512):
                            nc.tensor.matmul(
                                ps[:, cc * 512 : (cc + 1) * 512],
                                lw,
                                x[:, q0 + cc * 512 + j : q0 + cc * 512 + j + 512],
                                start=(j == 0),
                                stop=(j == K - 1),
                            )
                    nc.scalar.activation(
                        o[:, q0 : q0 + PQ], ps[:], mybir.ActivationFunctionType.Silu
                    )

            def dve_part(blk, x, o, t0, size, n_act=0):
                # Elementwise path for [t0, t0+size): qe holds the
                # even-shift products [q0 | q2], qo the odd [q1 | q3]
                # (n_act of the odd ones on ACT to relieve DVE), pair-add
                # + final add on DVE, then SiLU. Shift-rebased:
                # q_j[:, t] = w_j * x[:, t + j].
                wj = lambda j: w[:, blk * K + j : blk * K + j + 1]
                qe = pool.tile([128, 2, size], MID_DT, tag=f"qe{size}", bufs=3)
                qo = pool.tile([128, 2, size], MID_DT, tag=f"qo{size}", bufs=3)
                if n_act >= 1:
                    nc.scalar.mul(qo[:, 0, :], x[:, t0 + 1 : t0 + 1 + size], wj(1))
                else:
                    nc.vector.tensor_scalar_mul(
                        qo[:, 0, :], x[:, t0 + 1 : t0 + 1 + size], wj(1)
                    )
                nc.vector.tensor_scalar_mul(qe[:, 0, :], x[:, t0 : t0 + size], wj(0))
                if n_act >= 2:
                    nc.scalar.mul(qo[:, 1, :], x[:, t0 + 3 : t0 + 3 + size], wj(3))
                else:
                    nc.vector.tensor_scalar_mul(
                        qo[:, 1, :], x[:, t0 + 3 : t0 + 3 + size], wj(3)
                    )
                nc.vector.tensor_scalar_mul(qe[:, 1, :], x[:, t0 + 2 : t0 + 2 + size], wj(2))
                nc.vector.tensor_add(qe[:, :, :], qe[:, :, :], qo[:, :, :])
                # final add + SiLU in 1024-col pieces: keeps ACT's static
                # queue head short so PE PSUM drains never wait long
                for c0 in range(0, size, PQ):
                    cw = min(PQ, size - c0)
                    nc.vector.tensor_add(
                        qe[:, 0, c0 : c0 + cw],
                        qe[:, 0, c0 : c0 + cw],
                        qe[:, 1, c0 : c0 + cw],
                    )
                    nc.scalar.activation(
                        o[:, t0 + c0 : t0 + c0 + cw], qe[:, 0, c0 : c0 + cw],
                        mybir.ActivationFunctionType.Silu,
                    )

            # odd products on ACT per (blk, half): balances DVE ~40 / ACT ~40
            ACT_ODD = {(0, 0): 2, (0, 1): 1}
            mult = mybir.AluOpType.mult
            add = mybir.AluOpType.add
            # Block 7 is processed before block 6, and block 6's PE half
            # before its DVE half, so ACT's (statically ordered) SiLU queue
            # ends with exactly the last-ready work — otherwise the PE's
            # final PSUM quarters head-of-line block behind a late DVE SiLU.
            for blk in [0, 1, 2, 3, 4, 5, 7, 6]:
                x = xts[blk]
                o = pool.tile([128, L], MID_DT, tag="o", bufs=NBLK)
                halves = [1, 0] if blk == PE_HALF_BLK else [0, 1]
                for h in halves:
                    h0 = h * HALF
                    if blk in PE_BLKS or (blk == PE_HALF_BLK and h == 1):
                        pe_part(blk, x, o, h0, HALF)
                    elif blk == PE_HALF_BLK or blk == 0:
                        # 1024-col pieces: block 0 so compute ramps with
                        # the piecewise first load, block 6 so the final
                        # SiLU+store chain is short
                        n_act = ACT_ODD.get((blk, h), 0)
                        dve_part(blk, x, o, h0, PQ, n_act)
                        dve_part(blk, x, o, h0 + PQ, PQ, n_act)
                    else:
                        dve_part(blk, x, o, h0, HALF, ACT_ODD.get((blk, h), 0))
                    if blk != PE_HALF_BLK:
                        # Store gate: a no-op rewrite of the last output
                        # column that reads the last-loaded x tile. This
                        # gives the store a data dependency on the final
                        # load, so the dataflow scheduler cannot interleave
                        # store DMAs ahead of the last loads (which starves
                        # the PE and stretches the load phase).
                        e = h0 + HALF - 1
                        nc.vector.scalar_tensor_tensor(
                            o[:, e : e + 1], x_last[:, 0:1], 0.0,
                            o[:, e : e + 1], mult, add,
                        )
                    nc.sync.dma_start(
                        out=ot[blk * 128 : (blk + 1) * 128, h0 : h0 + HALF],
                        in_=o[:, h0 : h0 + HALF],
                    )
    nc.compile()
    return nc


def _shard_inputs(x, w):
    in_maps = []
    for core in range(N_CORES):
        b, half = divmod(core, 2)
        d0 = half * DH
        xt = np.zeros((DH, ROWW), dtype=np.float16)
        xt[:, PAD : PAD + L] = x[b, :, d0 : d0 + DH].T.astype(np.float16)
        # w rows for this shard, rearranged so partition p holds the K
        # weights of channel blk*128 + p at free cols [blk*K, blk*K + K)
        w_sh = w[d0 : d0 + DH].reshape(NBLK, 128, K)
        wt = (
            w_sh.transpose(1, 0, 2).reshape(128, NBLK * K).astype(np.float32)
        )
        # diag(w_j) slabs for the PE blocks: partition p, col j*128 + m
        # holds w[blk*128+p, j] iff m == p else 0
        wdv = np.zeros((128, len(PE_DIAG_BLKS), K, 128), dtype=np.float16)
        idx = np.arange(128)
        for i, blk in enumerate(PE_DIAG_BLKS):
            wdv[idx, i, :, idx] = w_sh[blk].astype(np.float16)
        in_maps.append(
            {
                "xt": np.ascontiguousarray(xt),
                "wt": np.ascontiguousarray(wt),
                "wd": np.ascontiguousarray(
                    wdv.reshape(128, len(PE_DIAG_BLKS) * K * 128)
                ),
            }
        )
    return in_maps


def kernel(x, w):
    x = np.asarray(x, dtype=np.float32)
    w = np.asarray(w, dtype=np.float32)
    assert x.shape == (B, L, D) and w.shape == (D, K)

    if "nc" not in _cache:
        _cache["nc"] = _build_bass()
    nc = _cache["nc"]

    in_maps = _shard_inputs(x, w)
    res = None
    for attempt in range(3):
        try:
            res = run_bass_kernel_spmd(nc, in_maps, core_ids=list(range(N_CORES)))
            break
        except Exception:
            if attempt == 2:
                raise
    _cache["last_results"] = res

    out = np.empty((B, L, D), dtype=np.float32)
    for core in range(N_CORES):
        b, half = divmod(core, 2)
        d0 = half * DH
        out[b, :, d0 : d0 + DH] = res.results[core]["ot"].T.astype(np.float32)
    return out
